# revision 20
# baseline (speedup 1.0000x reference)
"""AnalyticGaussianVelocity Trainium2 kernel, 8 NeuronCores.

Math (reference):
    a=t, b=1-t
    logit_n = -(1/(2b^2)) * (|x|^2 - 2a x.y_n + a^2 |y_n|^2)
    v = -(1/b) x + (1 + a/b) * softmax(logit) @ dataset

Device-side per core (dataset sharded along N, padded 50000->51200,
NSH=6400/core). N-outer two-pass (flash style): pass 0 covers n-chunks 0-5
for all 8 batch tiles, pass 1 covers chunks 6-12 — so compute starts as
soon as the first half of dsT lands instead of waiting for the full 13 MB
stream. Per (pass, tile) unit:
    G_n    = x.y_n - (a/2)(|y_n|^2 - 512)    (f32r matmuls + split-bf16 rank-1)
    copy+chunk-max fused on DVE (tensor_scalar accum=max)
    m_h    = c1 * rowmax(G over the pass-local chunks [pass 1: all chunks])
    P8_n   = e4m3(exp(c1*G - m_h + ln 64))   (Act exp -> fp8, f32 accum -> l)
    P8^T   = PE fp8 transposes (stride-2 psum out), u16 Act bank drains
    S_h    = P8 @ (ds_hi8 + ds_lo8)          (fp8e4 DoubleRow, 2x bf16 rate)
Pass 0's partial S is bounced to DRAM; pass 1 rescales and combines:
    f = exp(m0 - m);  S = S1*f + S2;  l = l0*f + l1      (DVE stt + Act exp)
Host combine across cores: w = exp(m_c - M); v = q1*x + q2*sum(wS)/sum(wl).
The 2^6 prescale cancels in S/l. Padding rows: |y|^2 = 2048^2*512 -> logit
-1e7 -> P=0; pad rows of ds_hi/lo are zeroed so 0*pad stays 0.
"""

import numpy as np
import ml_dtypes

import concourse.bass as bass
from concourse import bacc
import concourse.mybir as mybir
import concourse.tile as tile
from concourse.bass_utils import run_bass_kernel_spmd

F32 = mybir.dt.float32
F32R = mybir.dt.float32r
BF16 = mybir.dt.bfloat16
FP8 = mybir.dt.float8e4
U16 = mybir.dt.uint16
BF = ml_dtypes.bfloat16
E4 = ml_dtypes.float8_e4m3fn
DR = mybir.MatmulPerfMode.DoubleRow
X = mybir.AxisListType.X

B, D, N = 1024, 512, 50000
NCORES = 8
NPAD = 51200                      # 8 * 6400, multiple of 1024
NSH = NPAD // NCORES              # 6400 per core
KD = D // 128                     # 4 contraction tiles for logits matmul
BT = B // 128                     # 8 batch tiles
CHUNKS = [512] * 12 + [256]       # mm1 free-dim chunks (>=256: full-rate f32r)
NCH = len(CHUNKS)
HSPLIT = 6                        # pass 0 = chunks [0,6), pass 1 = [6,13)
GROUPS = NSH // 256               # 25 DoubleRow pair-groups per core
PADVAL = 2048.0
LN_SCALE = float(np.log(64.0))    # exp prescale 2^6 (cancels in S/l)
COFF = np.concatenate([[0], np.cumsum(CHUNKS)])
COFF0 = int(COFF[HSPLIT])         # n-offset where pass 1 starts (3072)
# DoubleRow groups covered by chunk c: (2c, 2c+1) for 512 chunks, (2c,) for 256
CGROUPS = [list(range(2 * c, 2 * c + (2 if CHUNKS[c] == 512 else 1)))
           for c in range(NCH)]


def _build():
    nc = bacc.Bacc("TRN2", target_bir_lowering=False, debug=False,
                   num_devices=NCORES, dynamic_dma_scratch_size=512)

    xT = nc.declare_dram_parameter("xT", [KD, 128, B], F32R, isOutput=False)
    dsT = nc.declare_dram_parameter("dsT", [KD, 128, NSH], F32R, isOutput=False)
    dhi = nc.declare_dram_parameter("ds_hi", [128, GROUPS, 2, D], FP8,
                                    isOutput=False)
    dlo = nc.declare_dram_parameter("ds_lo", [128, GROUPS, 2, D], FP8,
                                    isOutput=False)
    r1l = nc.declare_dram_parameter("r1_lhsT", [3, B], BF16, isOutput=False)
    r1r = nc.declare_dram_parameter("r1_rhs", [3, NSH], BF16, isOutput=False)
    c1d = nc.declare_dram_parameter("c1", [128, BT], F32, isOutput=False)
    idd = nc.declare_dram_parameter("ident", [128, 128], FP8, isOutput=False)
    yn1 = nc.declare_dram_parameter("ynorm1", [128, NSH - COFF0], F32,
                                    isOutput=False)
    uvd = nc.declare_dram_parameter("uv", [128, BT], F32, isOutput=False)
    S_out = nc.declare_dram_parameter("S_out", [B, D], F32, isOutput=True)
    m_out = nc.declare_dram_parameter("m_out", [B, 1], F32, isOutput=True)
    l_out = nc.declare_dram_parameter("l_out", [B, 1], F32, isOutput=True)
    S_loc = nc.dram_tensor("S_loc", [B, D], F32)

    with tile.TileContext(nc) as tc:
        with (
            tc.tile_pool(name="res", bufs=1) as res,
            tc.tile_pool(name="gpool", bufs=15) as gpool,
            tc.tile_pool(name="ppool", bufs=3) as ppool,
            tc.tile_pool(name="small", bufs=2) as small,
            tc.tile_pool(name="ptsb", bufs=2) as ptsb_pool,
            tc.tile_pool(name="sbout", bufs=1) as sbout,
            tc.tile_pool(name="s1pool", bufs=2) as s1pool,
            tc.tile_pool(name="bigA", bufs=1) as bigA,
            tc.tile_pool(name="gps", bufs=4, space="PSUM") as gps,
            tc.tile_pool(name="spsum", bufs=2, space="PSUM") as spsum,
            tc.tile_pool(name="ptps", bufs=2, space="PSUM") as ptps,
        ):
            # ---- residents; smalls on the gpsimd queue ----
            r1l_t = res.tile([3, B], BF16, tag="r1l")
            nc.gpsimd.dma_start(r1l_t[:], r1l[:])
            r1r_t = res.tile([3, NSH], BF16, tag="r1r")
            nc.gpsimd.dma_start(r1r_t[:], r1r[:])
            c1_t = res.tile([128, BT], F32, tag="c1")
            nc.gpsimd.dma_start(c1_t[:], c1d[:])
            id_t = res.tile([128, 128], FP8, tag="ident")
            nc.gpsimd.dma_start(id_t[:], idd[:])

            # big streams, demand-ordered on the sync queue: xT b0/b1,
            # dsT pass-0 chunks (with xT and the pass-0 ds_hi/lo groups
            # woven in), then pass-1 chunks and groups.
            xT_r = res.tile([128, KD, B], F32R, tag="xT_r")
            xT_re = xT.ap().rearrange("k p b -> p k b")
            dsT_A = bigA.tile([128, KD, COFF0], F32R, tag="big")
            dsT_B = res.tile([128, KD, NSH - COFF0], F32R, tag="dsT_B")
            c1u_t = res.tile([128, BT], F32, tag="uv")
            nc.gpsimd.dma_start(c1u_t[:], uvd[:])
            dhi_t = res.tile([128, GROUPS, 2, D], FP8, tag="dhi")
            dlo_t = res.tile([128, GROUPS, 2, D], FP8, tag="dlo")

            def ds_slice(k, o, w):
                if o < COFF0:
                    return dsT_A[:, k, o:o + w]
                return dsT_B[:, k, o - COFF0:o - COFF0 + w]

            def dma_dsT_chunk(c):
                o = int(COFF[c])
                w = CHUNKS[c]
                for k in range(KD):
                    nc.sync.dma_start(ds_slice(k, o, w),
                                      dsT.ap()[k, :, o:o + w])

            def dma_xt(i):
                nc.sync.dma_start(xT_r[:, :, i * 128:(i + 1) * 128],
                                  xT_re[:, :, i * 128:(i + 1) * 128])

            def dma_ds8(g0, g1):
                nc.sync.dma_start(dhi_t[:, g0:g1], dhi.ap()[:, g0:g1])
                nc.sync.dma_start(dlo_t[:, g0:g1], dlo.ap()[:, g0:g1])

            dma_xt(0)
            dma_dsT_chunk(0)
            dma_xt(1)
            dma_dsT_chunk(1)
            dma_dsT_chunk(2)
            dma_xt(2)
            dma_dsT_chunk(3)
            dma_xt(3)
            dma_dsT_chunk(4)
            dma_dsT_chunk(5)
            dma_ds8(0, 4)
            for i in range(4, BT):
                dma_xt(i)
            dma_ds8(4, 12)
            for c in range(HSPLIT, NCH):
                dma_dsT_chunk(c)
                if c == HSPLIT + 1:
                    dma_ds8(12, 18)
            dma_ds8(18, GROUPS)

            gmax_r = res.tile([128, BT * NCH], F32, tag="gmax_r")
            m0_sb = res.tile([128, BT], F32, tag="m0_sb")
            l0_sb = res.tile([128, BT], F32, tag="l0_sb")
            m_sb = res.tile([128, BT], F32, tag="m_sb")
            l_sb = res.tile([128, BT], F32, tag="l_sb")

            # warm the pt psum bufs: defines the odd fp8 lanes the stride-2
            # transposes skip, so full-bank u16 drains read initialized bytes
            for _ in range(2):
                pt_ps = ptps.tile([128, 512], F32, tag="ptb")
                nc.vector.memset(pt_ps[:], 0.0)

            # ---- units: (pass h, tile i, chunk list) ----
            units = []
            for h, (ca, cb) in enumerate(((0, HSPLIT), (HSPLIT, NCH))):
                for i in range(BT):
                    units.append((h, i, list(range(ca, cb))))

            st = [dict() for _ in units]  # per-unit tiles keyed by role

            def emit_mm1(u, c):
                h, i, _ = units[u]
                w = CHUNKS[c]
                o = int(COFF[c])
                g_ps = gps.tile([128, 512], F32, tag="gps")
                for k in range(KD):
                    nc.tensor.matmul(
                        g_ps[:, :w],
                        xT_r[:, k, i * 128:(i + 1) * 128],
                        ds_slice(k, o, w),
                        start=(k == 0), stop=(h == 1 and k == KD - 1),
                    )
                G_c = gpool.tile([128, 512], F32, tag="G")
                if h == 0:
                    # rank-1 |y|^2 term on PE (split-bf16), then fused
                    # PSUM->SBUF copy + chunk rowmax on DVE
                    nc.tensor.matmul(
                        g_ps[:, :w],
                        r1l_t[:, i * 128:(i + 1) * 128],
                        r1r_t[:, o:o + w],
                        start=False, stop=True,
                    )
                    nc.vector.tensor_scalar(
                        out=G_c[:, :w], in0=g_ps[:, :w], scalar1=1.0,
                        scalar2=None, op0=mybir.AluOpType.mult,
                        op1=mybir.AluOpType.max,
                        accum_out=gmax_r[:, i * NCH + c:i * NCH + c + 1])
                else:
                    # rank-1 on DVE: G = u_i * ynorm + x.y (exact f32), then
                    # chunk rowmax; frees the PE rank-1 pass
                    oo = o - COFF0
                    nc.vector.scalar_tensor_tensor(
                        out=G_c[:, :w], in0=yn_t[0][:, oo:oo + w],
                        scalar=c1u_t[:, i:i + 1], in1=g_ps[:, :w],
                        op0=mybir.AluOpType.mult, op1=mybir.AluOpType.add)
                    nc.vector.reduce_max(
                        gmax_r[:, i * NCH + c:i * NCH + c + 1], G_c[:, :w],
                        axis=X, op=mybir.AluOpType.max)
                st[u][("G", c)] = G_c

            def emit_head(u):
                h, i, chunks = units[u]
                gm = small.tile([128, 1], F32, tag="gm")
                if h == 0:
                    cols = gmax_r[:, i * NCH:i * NCH + HSPLIT]
                else:
                    cols = gmax_r[:, i * NCH:i * NCH + NCH]
                nc.vector.reduce_max(gm[:], cols, axis=X,
                                     op=mybir.AluOpType.max)
                mdst = m0_sb if h == 0 else m_sb
                nc.vector.tensor_mul(mdst[:, i:i + 1], gm[:], c1_t[:, i:i + 1])
                nb = small.tile([128, 1], F32, tag="nb")
                nc.vector.tensor_scalar(
                    out=nb[:], in0=mdst[:, i:i + 1], scalar1=-1.0,
                    scalar2=LN_SCALE, op0=mybir.AluOpType.mult,
                    op1=mybir.AluOpType.add)
                if h == 1:
                    nc.sync.dma_start(m_out[i * 128:(i + 1) * 128, :],
                                      m_sb[:, i:i + 1])
                    # prefetch pass-0 partial S for the tail combine
                    S1 = s1pool.tile([128, D], F32, tag="S1")
                    nc.sync.dma_start(S1[:], S_loc[i * 128:(i + 1) * 128, :])
                    st[u]["S1"] = S1
                    # rescale factor f = exp(m0 - m) is known already
                    d = small.tile([128, 1], F32, tag="d")
                    nc.vector.tensor_sub(d[:], m0_sb[:, i:i + 1],
                                         m_sb[:, i:i + 1])
                    f = small.tile([128, 1], F32, tag="f")
                    nc.scalar.activation(f[:], d[:],
                                         mybir.ActivationFunctionType.Exp)
                    st[u]["f"] = f
                lp = small.tile([128, len(chunks)], F32, tag="lp")
                S_ps = spsum.tile([128, D], F32, tag="S")
                st[u]["nb"] = nb
                st[u]["lp"] = lp
                st[u]["S"] = S_ps

            def emit_exp(u, c):
                h, i, chunks = units[u]
                w = CHUNKS[c]
                G_c = st[u].pop(("G", c))
                P_c = ppool.tile([128, 512], FP8, tag="P")
                nc.scalar.activation(
                    P_c[:, :w], G_c[:, :w],
                    mybir.ActivationFunctionType.Exp,
                    bias=st[u]["nb"][:], scale=c1_t[:, i:i + 1],
                    accum_out=st[u]["lp"][:, c - chunks[0]:c - chunks[0] + 1],
                )
                st[u][("P", c)] = P_c

            def emit_tp(u, c):
                h, i, chunks = units[u]
                w = CHUNKS[c]
                lb = (c - chunks[0]) // 2
                if (c - chunks[0]) % 2 == 0:
                    pt_ps = ptps.tile([128, 512], F32, tag="ptb")
                    st[u][("B", lb)] = pt_ps
                pt_ps = st[u][("B", lb)]
                pt8 = pt_ps[:].bitcast(FP8).rearrange(
                    "p (t m two) -> p t m two", t=8, two=2)
                P_c = st[u].pop(("P", c))
                t0 = ((c - chunks[0]) % 2) * 4
                nt = w // 128
                last_c = (c == chunks[-1])
                for j in range(nt):
                    first = ((c - chunks[0]) % 2 == 0) and (j == 0)
                    last = (j == nt - 1) and ((c - chunks[0]) % 2 == 1 or last_c)
                    nc.tensor.matmul(pt8[:, t0 + j, :, 0],
                                     P_c[:, j * 128:(j + 1) * 128],
                                     id_t[:], is_transpose=True,
                                     start=first, stop=last)

            def emit_drain(u, lb):
                pt_ps = st[u].pop(("B", lb))
                pt_sb = ptsb_pool.tile([128, 8, 128, 2], FP8, tag="ptsb")
                if units[u][0] == 0 or u == len(units) - 1:
                    nc.vector.tensor_copy(pt_sb[:].bitcast(U16),
                                          pt_ps[:].bitcast(U16))
                else:
                    nc.scalar.activation(pt_sb[:].bitcast(U16),
                                         pt_ps[:].bitcast(U16),
                                         mybir.ActivationFunctionType.Copy)
                st[u][("T", lb)] = pt_sb

            def emit_mm2(u, lb):
                h, i, chunks = units[u]
                pt_sb = st[u].pop(("T", lb))
                cs = chunks[2 * lb:2 * lb + 2]
                groups = [g for c in cs for g in CGROUPS[c]]
                g_start = CGROUPS[chunks[0]][0]
                g_end = CGROUPS[chunks[-1]][-1]
                for p, g in enumerate(groups):
                    for hl, dn in ((0, dhi_t), (1, dlo_t)):
                        nc.tensor.matmul(
                            st[u]["S"][:], pt_sb[:, 2 * p:2 * p + 2, :, 0],
                            dn[:, g, :, :],
                            start=(g == g_start and hl == 0),
                            stop=(g == g_end and hl == 1),
                            perf_mode=DR)

            def emit_tail(u):
                h, i, chunks = units[u]
                S_ps = st[u].pop("S")
                lp = st[u].pop("lp")
                if h == 0:
                    nc.vector.reduce_sum(l0_sb[:, i:i + 1], lp[:], axis=X,
                                         op=mybir.AluOpType.add)
                    S_sb = sbout.tile([128, D], F32, tag="S_sb")
                    nc.vector.tensor_copy(S_sb[:], S_ps[:])
                    nc.sync.dma_start(S_loc[i * 128:(i + 1) * 128, :], S_sb[:])
                else:
                    l1 = small.tile([128, 1], F32, tag="l1")
                    nc.vector.reduce_sum(l1[:], lp[:], axis=X,
                                         op=mybir.AluOpType.add)
                    f = st[u].pop("f")
                    nc.vector.scalar_tensor_tensor(
                        out=l_sb[:, i:i + 1], in0=l0_sb[:, i:i + 1],
                        scalar=f[:], in1=l1[:],
                        op0=mybir.AluOpType.mult, op1=mybir.AluOpType.add)
                    nc.sync.dma_start(l_out[i * 128:(i + 1) * 128, :],
                                      l_sb[:, i:i + 1])
                    S_sb = sbout.tile([128, D], F32, tag="S_sb")
                    nc.vector.scalar_tensor_tensor(
                        out=S_sb[:], in0=st[u].pop("S1")[:], scalar=f[:],
                        in1=S_ps[:],
                        op0=mybir.AluOpType.mult, op1=mybir.AluOpType.add)
                    nc.sync.dma_start(S_out[i * 128:(i + 1) * 128, :], S_sb[:])

            # ---- software pipeline: zip the mm1 cursor with the lagged
            # softmax event stream; gates keep a unit's softmax behind its
            # last mm1, the G-in-flight cap keeps the cursor from deadlocking
            # the DVE queue on gpool exhaustion.
            mm1_list = [(u, c) for u, (h, i, chunks) in enumerate(units)
                        for c in chunks]
            mm1_end = {}
            for idx, (u, c) in enumerate(mm1_list):
                mm1_end[u] = idx + 1

            soft_list = []
            for u, (h, i, chunks) in enumerate(units):
                ev = [("head", u, 0), ("exp", u, chunks[0])]
                nloc = len(chunks)
                pend = []
                for k, c in enumerate(chunks):
                    if k + 1 < nloc:
                        ev.append(("exp", u, chunks[k + 1]))
                    ev.append(("tp", u, c))
                    if k % 2 == 1 or k == nloc - 1:
                        ev.append(("drain", u, k // 2))
                        pend.append(k // 2)
                    if len(pend) > 1:
                        ev.append(("mm2", u, pend.pop(0)))
                while pend:
                    ev.append(("mm2", u, pend.pop(0)))
                ev.append(("tail", u, 0))
                soft_list.extend(ev)

            GCAP = 13                 # max G tiles in flight (gpool - 2)
            N_P0 = BT * HSPLIT        # mm1 count in pass 0
            yn_t = [None]
            mi = si = 0
            g_live = 0
            while mi < len(mm1_list) or si < len(soft_list):
                progressed = False
                if mi < len(mm1_list) and g_live < GCAP:
                    u, c = mm1_list[mi]
                    emit_mm1(u, c)
                    mi += 1
                    g_live += 1
                    progressed = True
                    if mi == N_P0:
                        # pass-0 dsT region is done after these mm1s; alias
                        # it with the replicated |y|^2 row block for the
                        # pass-1 DVE rank-1 (WAR dep handled by the pool)
                        ynt = bigA.tile([128, KD * COFF0], F32, tag="big")
                        nc.gpsimd.dma_start(
                            ynt[:, :NSH - COFF0], yn1[:])
                        yn_t[0] = ynt
                budget = 4 if mi < len(mm1_list) else 10 ** 9
                n_emit = 0
                while si < len(soft_list) and n_emit < budget:
                    kind, u, arg = soft_list[si]
                    if mm1_end[u] > mi:
                        break
                    if kind == "head":
                        emit_head(u)
                    elif kind == "exp":
                        emit_exp(u, arg)
                        g_live -= 1
                    elif kind == "tp":
                        emit_tp(u, arg)
                    elif kind == "drain":
                        emit_drain(u, arg)
                    elif kind == "mm2":
                        emit_mm2(u, arg)
                    elif kind == "tail":
                        emit_tail(u)
                    si += 1
                    n_emit += 1
                    progressed = True
                if not progressed:
                    raise RuntimeError("pipeline emission wedged")

    nc.compile()
    return nc


_NC_CACHE = {}


def _get_nc(combine=False):
    if "nc" not in _NC_CACHE:
        _NC_CACHE["nc"] = _build()
    return _NC_CACHE["nc"]


def _split_bf16(v):
    hi = v.astype(np.float32).astype(BF)
    lo = (v.astype(np.float64) - hi.astype(np.float64)).astype(np.float32).astype(BF)
    return hi, lo


def _split_e4(v):
    hi = v.astype(np.float32).astype(E4)
    lo = (v.astype(np.float64) - hi.astype(np.float64)).astype(np.float32).astype(E4)
    return hi, lo


def _prep_inputs(x_t, t, dataset, combine=False):
    x_t = np.asarray(x_t, dtype=np.float32)
    t = np.asarray(t, dtype=np.float32)
    dataset = np.asarray(dataset, dtype=np.float32)

    a = t.astype(np.float64)
    b = 1.0 - a
    c1 = np.ascontiguousarray(
        (a / (b * b)).astype(np.float32).reshape(BT, 128).T)
    u = -a / 2.0
    u_hi, u_lo = _split_bf16(u)
    r1_lhsT = np.stack([u_hi, u_lo, u_hi]).astype(BF)          # (3, B)
    uv = np.ascontiguousarray(
        u.astype(np.float32).reshape(BT, 128).T)               # (128, BT)

    dsp = np.full((NPAD, D), PADVAL, dtype=np.float32)
    dsp[:N] = dataset
    dsnc = (dsp.astype(np.float64) ** 2).sum(1) - float(D)      # centered |y|^2
    v_hi, v_lo = _split_bf16(dsnc)
    r1_rhs_full = np.stack([v_hi, v_hi, v_lo]).astype(BF)       # (3, NPAD)

    xT = np.ascontiguousarray(x_t.T).reshape(KD, 128, B)
    dsT_full = np.ascontiguousarray(dsp.T)                      # (D, NPAD)

    dsz = dsp.copy()
    dsz[N:] = 0.0                                               # pad rows -> 0
    ds_hi, ds_lo = _split_e4(dsz)

    ident = np.eye(128, dtype=np.float32).astype(E4)

    in_maps = []
    for c in range(NCORES):
        sl = slice(c * NSH, (c + 1) * NSH)
        # [128(k), GROUPS, 2, D]: element (k, g, i, d) = ds[(2g+i)*128+k, d]
        hi_c = np.ascontiguousarray(
            ds_hi[sl].reshape(GROUPS, 2, 128, D).transpose(2, 0, 1, 3))
        lo_c = np.ascontiguousarray(
            ds_lo[sl].reshape(GROUPS, 2, 128, D).transpose(2, 0, 1, 3))
        yn_row = dsnc[sl][COFF0:].astype(np.float32)
        im = {
            "xT": xT,
            "dsT": np.ascontiguousarray(dsT_full[:, sl]).reshape(KD, 128, NSH),
            "ds_hi": hi_c,
            "ds_lo": lo_c,
            "r1_lhsT": r1_lhsT,
            "r1_rhs": np.ascontiguousarray(r1_rhs_full[:, sl]),
            "c1": c1,
            "ident": ident,
            "ynorm1": np.ascontiguousarray(
                np.broadcast_to(yn_row, (128, NSH - COFF0))),
            "uv": uv,
        }
        in_maps.append(im)
    return in_maps


def _combine_host(results, x_t, t):
    a = t.astype(np.float64)
    b = 1.0 - a
    m_c = np.stack([np.asarray(r["m_out"], dtype=np.float64)[:, 0]
                    for r in results])                          # (8, B)
    l_c = np.stack([np.asarray(r["l_out"], dtype=np.float64)[:, 0]
                    for r in results])                          # (8, B)
    S_c = np.stack([np.asarray(r["S_out"], dtype=np.float64)
                    for r in results])                          # (8, B, D)
    M = m_c.max(0)
    w = np.exp(m_c - M)                                         # (8, B)
    S = np.einsum("cb,cbd->bd", w, S_c)
    L = (w * l_c).sum(0)
    wd = S / L[:, None]
    v = (-1.0 / b)[:, None] * x_t.astype(np.float64) \
        + (1.0 + a / b)[:, None] * wd
    return v.astype(np.float32)


def run_full(x_t, t, dataset, trace=False, combine=False):
    nc = _get_nc()
    in_maps = _prep_inputs(x_t, t, dataset)
    res = run_bass_kernel_spmd(nc, in_maps, core_ids=list(range(NCORES)),
                               trace=trace)
    v = _combine_host(res.results, np.asarray(x_t, np.float32),
                      np.asarray(t, np.float32))
    return v, res


def kernel(x_t, t, dataset):
    v, _ = run_full(x_t, t, dataset)
    return v
